# revision 12
# baseline (speedup 1.0000x reference)
"""Single-head attention (B=4, S=2048, D=1024) on 8 TRN2 NeuronCores.

Sharding: each core handles one (batch, query-half) pair -> 8 shards of
1024 query rows. K/V projections are split between the two cores of a
batch pair (each projects its own 1024-row sequence half) and exchanged
with 2-rank AllGathers, overlapped with the V/Q projections.

v2 pipeline changes vs baseline:
  - PE warm-up: dummy matmuls issued at t=0 fill the input-DMA head so the
    HAM clock gate reaches 8/8 before real matmuls arrive.
  - projection order K -> V -> Q: V-gathers trigger early so the gathered V
    lands before the AV phase needs it; scores (which need gathered K) run
    after Q proj anyway.
  - K is exchanged in fp8e4 (half the wire bytes) because the scores matmul
    consumes it in fp8 anyway; 1 chunk. V stays bf16, 4 chunks.
  - gpsimd is a pure collective queue (collective_compute only exists
    there); bounce writes go on scalar, reloads on sync, so collective
    completion waits never block evictions/compute.
  - scores matmul in fp8 DoubleRow (2 contraction tiles per instruction):
    q_proj/k_proj quantized at natural scale (sigma ~0.6), the 1/32
    attention scale is folded into the exp() activation's scale operand.
  - output written bf16 (host upcasts); AV k-order permuted to match
    V-gather chunk arrival.
Compute dtype bf16 (scores fp8-DR), fp32 PSUM accumulation.
"""

import sys

import numpy as np

try:
    import concourse  # noqa: F401
except ImportError:  # pragma: no cover
    sys.path.insert(0, "/opt/trn_rl_repo")

import ml_dtypes

import concourse.bass as bass  # noqa: F401
import concourse.mybir as mybir
import concourse.tile as tile
from concourse import bacc
from concourse.bass import ds, ts
from concourse.bass_utils import run_bass_kernel_spmd

P = 128          # partitions
D = 1024         # embed dim
S = 2048         # sequence length
B = 4            # batch
QH = S // 2      # query/sequence rows per core
NCORES = 8
DJ = D // P      # 8  d-tiles
KJ = S // P      # 16 k/s-tiles
HJ = KJ // 2     # 8  s-tiles per half
QJ = QH // P     # 8  q-tiles
NCH = 512        # moving-operand chunk (one PSUM bank of fp32)
ISCALE = 1.0 / 32.0  # 1/sqrt(D), folded into the exp activation
NWARM = 10       # PE warm-up matmuls (~3.5us busy: HAM hot by then)

DT = mybir.dt.bfloat16
F8 = mybir.dt.float8e4
F32 = mybir.dt.float32
NPDT = ml_dtypes.bfloat16

AF = mybir.ActivationFunctionType
OP = mybir.AluOpType
DR = mybir.MatmulPerfMode.DoubleRow

PAIRS = [[0, 1], [2, 3], [4, 5], [6, 7]]

# AV contraction order: V-gather chunk c delivers global k-tiles
# {2c, 2c+1, 8+2c, 8+2c+1}; consume in arrival order.
KT_ORDER = [kt for c in range(HJ // 2)
            for kt in (2 * c, 2 * c + 1, HJ + 2 * c, HJ + 2 * c + 1)]


def build():
    nc = bacc.Bacc("TRN2", target_bir_lowering=False, debug=False,
                   num_devices=NCORES)

    qT_d = nc.dram_tensor("qT", [D, QH], DT, kind="ExternalInput").ap()
    kT_d = nc.dram_tensor("kT", [D, QH], DT, kind="ExternalInput").ap()
    vT_d = nc.dram_tensor("vT", [D, QH], DT, kind="ExternalInput").ap()
    # wq/wk pre-sliced by output tile on the host: [do, d_in, 128]
    wq_d = nc.dram_tensor("wq", [DJ, P, D], DT, kind="ExternalInput").ap()
    wk_d = nc.dram_tensor("wk", [DJ, P, D], DT, kind="ExternalInput").ap()
    wv_d = nc.dram_tensor("wv", [D, D], DT, kind="ExternalInput").ap()
    bq_d = nc.dram_tensor("bqc", [P, DJ], F32, kind="ExternalInput").ap()
    bk_d = nc.dram_tensor("bkc", [P, DJ], F32, kind="ExternalInput").ap()
    bv_d = nc.dram_tensor("bvb", [P, D], DT, kind="ExternalInput").ap()
    out_d = nc.dram_tensor("out", [QH, D], DT, kind="ExternalOutput").ap()

    def part3(ap):  # [(n p), d] -> [p, n, d]
        return ap.rearrange("(n p) d -> p n d", p=P)

    with tile.TileContext(nc) as tc:
        with (
            tc.tile_pool(name="persist", bufs=1) as pp,
            tc.tile_pool(name="xin", bufs=16) as xp,
            tc.tile_pool(name="win", bufs=8) as wp,
            tc.tile_pool(name="ev", bufs=3) as ep,
            tc.tile_pool(name="psum", bufs=3, space="PSUM") as psp,
            tc.tile_pool(name="dram", bufs=1, space="DRAM") as dp,
        ):
            # collective bounce buffers (internal DRAM)
            KCH = 8  # K gather: one fp8 collective (1MB send)
            kbc = [dp.tile([KCH * P, QH], F8, tag=f"kb{c}", name=f"kb{c}")
                   for c in range(DJ // KCH)]
            kgc = [dp.tile([2, KCH * P, QH], F8, tag=f"kg{c}", name=f"kg{c}")
                   for c in range(DJ // KCH)]
            vbc = [dp.tile([2 * P, D], DT, tag=f"vb{c}", name=f"vb{c}")
                   for c in range(HJ // 2)]
            vgc = [dp.tile([2, 2 * P, D], DT, tag=f"vg{c}", name=f"vg{c}")
                   for c in range(HJ // 2)]

            # constants + warm-up scratch (vector queue)
            ones_t = pp.tile([P, 1], DT, tag="ones")
            nc.vector.memset(ones_t[:], 1.0)
            warm_t = pp.tile([P, NCH], DT, tag="warm")
            nc.vector.memset(warm_t[:], 0.25)

            # PE warm-up: garbage matmuls with no input deps beyond the memset
            for i in range(NWARM):
                psw = psp.tile([P, NCH], F32, tag="psA")
                nc.tensor.matmul(psw[:], warm_t[:, ds(0, P)], warm_t[:],
                                 start=True, stop=True)

            # ---- all input/weight loads issued up front
            # sync queue: x inputs (K first, then V, then Q), then K gathers
            def load_x(src, tag="x", split=0, eng=None, stripe=False):
                eng = eng or nc.sync
                out = []
                for di in range(DJ):
                    t = xp.tile([P, QH], DT, tag=tag)
                    # striped halves: both rings -> all 16 DMA engines pull
                    # this tensor ahead of anything emitted later
                    e2 = nc.scalar if stripe else eng
                    if di < split:
                        eng.dma_start(t[:, ds(0, NCH)],
                                      src[:, di, ds(0, NCH)])
                        e2.dma_start(t[:, ds(NCH, NCH)],
                                     src[:, di, ds(NCH, NCH)])
                    else:
                        eng.dma_start(t[:], src[:, di, :])
                    out.append(t)
                return out

            kT_in = load_x(part3(kT_d), tag="xk", split=DJ, stripe=True)
            vT_in = load_x(part3(vT_d))
            # qT loads are emitted after the K-projection loop so the K
            # bounce-eviction DMAs aren't queued behind them on the sync ring

            # scalar queue: weights (K first, then V, then Q)
            def load_w2(src):
                out = []
                for do in range(DJ):
                    t = wp.tile([P, DJ, P], DT, tag="w")
                    if do == 0:
                        nc.scalar.dma_start(t[:, ds(0, DJ // 2), :],
                                            src[do][:, ds(0, D // 2)])
                        nc.scalar.dma_start(t[:, ds(DJ // 2, DJ // 2), :],
                                            src[do][:, ds(D // 2, D // 2)])
                    else:
                        nc.scalar.dma_start(t[:], src[do])
                    out.append(t)
                return out

            bk_t = pp.tile([P, DJ], F32, tag="bk")
            nc.scalar.dma_start(bk_t[:], bk_d[:])
            bq_t = pp.tile([P, DJ], F32, tag="bq")
            nc.scalar.dma_start(bq_t[:], bq_d[:])
            wk_t = load_w2(wk_d)

            def load_w(src):
                out = []
                for di in range(DJ):
                    t = wp.tile([P, D], DT, tag="wv")
                    nc.scalar.dma_start(t[:], src[:, di, :])
                    out.append(t)
                return out

            wv_t = load_w(part3(wv_d))


            # persistent intermediates
            qT_proj = pp.tile([P, DJ, QH], F8, tag="qproj")   # (Q+bq)^T
            expT = pp.tile([P, KJ, QH], DT, tag="expT")       # exp(scores)^T
            v_full = pp.tile([P, KJ, D], DT, tag="vfull")     # gathered V
            # gathered K^T in fp8, grouped by do-PAIR for DoubleRow:
            # kf[p] = [128, 2, S] covering do = 2p, 2p+1
            kT_f = [pp.tile([P, 2, S], F8, tag=f"kf{p}", name=f"kf{p}")
                    for p in range(DJ // 2)]

            # ---- K projection (own half first; evict fp8 -> bounce)
            for do in range(DJ):
                ps0 = psp.tile([P, NCH], F32, tag="psA")
                ps1 = psp.tile([P, NCH], F32, tag="psB")
                for di in range(DJ):
                    w_ap = wk_t[do][:, di, :]
                    nc.tensor.matmul(ps0[:], w_ap, kT_in[di][:, ds(0, NCH)],
                                     start=(di == 0), stop=(di == DJ - 1))
                    nc.tensor.matmul(ps1[:], w_ap, kT_in[di][:, ds(NCH, NCH)],
                                     start=(di == 0), stop=(di == DJ - 1))
                ev = ep.tile([P, D], F8, tag="evk", bufs=4)
                nc.vector.tensor_scalar_add(ev[:, ds(0, NCH)], ps0[:],
                                            bk_t[:, ds(do, 1)])
                nc.vector.tensor_scalar_add(ev[:, ds(NCH, NCH)], ps1[:],
                                            bk_t[:, ds(do, 1)])
                nc.sync.dma_start(kbc[do // KCH][ts(do % KCH, P), :], ev[:])
                if do % KCH == KCH - 1:
                    c = do // KCH
                    nc.gpsimd.collective_compute(
                        "AllGather", OP.bypass, replica_groups=PAIRS,
                        ins=[kbc[c].opt()], outs=[kgc[c].opt()])
                    # reload right behind the gather on the same queue
                    for dd in range(KCH):
                        dog = c * KCH + dd
                        for g in range(2):
                            nc.sync.dma_start(
                                kT_f[dog // 2][:, dog % 2, ds(g * QH, QH)],
                                kgc[c][g, ts(dd, P), :])

            qT_in = load_x(part3(qT_d), eng=nc.scalar)

            # ---- V projection (own half, natural layout, no bias)
            for st in range(HJ):
                ps0 = psp.tile([P, NCH], F32, tag="psA")
                ps1 = psp.tile([P, NCH], F32, tag="psB")
                for di in range(DJ):
                    v_ap = vT_in[di][:, ts(st, P)]
                    nc.tensor.matmul(ps0[:], v_ap, wv_t[di][:, ds(0, NCH)],
                                     start=(di == 0), stop=(di == DJ - 1))
                    nc.tensor.matmul(ps1[:], v_ap, wv_t[di][:, ds(NCH, NCH)],
                                     start=(di == 0), stop=(di == DJ - 1))
                ev = ep.tile([P, D], DT, tag="ev")
                nc.vector.tensor_copy(ev[:, ds(0, NCH)], ps0[:])
                nc.vector.tensor_copy(ev[:, ds(NCH, NCH)], ps1[:])
                nc.scalar.dma_start(vbc[st // 2][ts(st % 2, P), :], ev[:])
                if st % 2 == 1:
                    c = st // 2
                    nc.gpsimd.collective_compute(
                        "AllGather", OP.bypass, replica_groups=PAIRS,
                        ins=[vbc[c].opt()], outs=[vgc[c].opt()])
                    # reload the 4 global k-tiles this chunk delivers
                    for g in range(2):
                        for j in range(2):
                            kt = g * HJ + 2 * c + j
                            nc.sync.dma_start(v_full[:, kt, :],
                                              vgc[c][g, ts(j, P), :])

            # wq loads reuse wk's pool slots (freed ~36us, needed ~63us);
            # emitted here so the bounce-eviction DMAs above aren't stuck
            # behind their WAR waits on the scalar queue
            wq_t = load_w2(wq_d)
            bv_t = pp.tile([P, D], DT, tag="bv")
            nc.scalar.dma_start(bv_t[:], bv_d[:])

            # ---- Q projection -> qT_proj fp8 [d_out, q] (natural scale)
            for do in range(DJ):
                ps0 = psp.tile([P, NCH], F32, tag="psA")
                ps1 = psp.tile([P, NCH], F32, tag="psB")
                for di in range(DJ):
                    w_ap = wq_t[do][:, di, :]
                    nc.tensor.matmul(ps0[:], w_ap, qT_in[di][:, ds(0, NCH)],
                                     start=(di == 0), stop=(di == DJ - 1))
                    nc.tensor.matmul(ps1[:], w_ap, qT_in[di][:, ds(NCH, NCH)],
                                     start=(di == 0), stop=(di == DJ - 1))
                nc.vector.tensor_scalar_add(qT_proj[:, do, ds(0, NCH)],
                                            ps0[:], bq_t[:, ds(do, 1)])
                nc.vector.tensor_scalar_add(qT_proj[:, do, ds(NCH, NCH)],
                                            ps1[:], bq_t[:, ds(do, 1)])

            # ---- scores^T + exp -> expT [k, q]  (fp8 DoubleRow)
            for kt in range(KJ):
                ps0 = psp.tile([P, NCH], F32, tag="psA")
                ps1 = psp.tile([P, NCH], F32, tag="psB")
                for c in range(DJ // 2):
                    k_ap = kT_f[c][:, :, ts(kt, P)]
                    nc.tensor.matmul(ps0[:], k_ap,
                                     qT_proj[:, ds(2 * c, 2), ds(0, NCH)],
                                     start=(c == 0), stop=(c == DJ // 2 - 1),
                                     perf_mode=DR)
                    nc.tensor.matmul(ps1[:], k_ap,
                                     qT_proj[:, ds(2 * c, 2), ds(NCH, NCH)],
                                     start=(c == 0), stop=(c == DJ // 2 - 1),
                                     perf_mode=DR)
                nc.scalar.activation(expT[:, kt, ds(0, NCH)], ps0[:], AF.Exp,
                                     scale=ISCALE)
                nc.scalar.activation(expT[:, kt, ds(NCH, NCH)], ps1[:],
                                     AF.Exp, scale=ISCALE)

            # ---- AV + fused normalize/bias -> out (bf16)
            for qt in range(QJ):
                po0 = psp.tile([P, NCH], F32, tag="psA")
                po1 = psp.tile([P, NCH], F32, tag="psB")
                psm = psp.tile([P, 1], F32, tag="psS", bufs=2)
                for i, kt in enumerate(KT_ORDER):
                    e_ap = expT[:, kt, ts(qt, P)]
                    nc.tensor.matmul(po0[:], e_ap, v_full[:, kt, ds(0, NCH)],
                                     start=(i == 0), stop=(i == KJ - 1))
                    nc.tensor.matmul(po1[:], e_ap, v_full[:, kt, ds(NCH, NCH)],
                                     start=(i == 0), stop=(i == KJ - 1))
                    nc.tensor.matmul(psm[:], e_ap, ones_t[:],
                                     start=(i == 0), stop=(i == KJ - 1))
                recip = ep.tile([P, 1], F32, tag="recip")
                nc.vector.reciprocal(recip[:], psm[:])
                ot = ep.tile([P, D], DT, tag="out", bufs=2)
                for h in range(4):
                    po = po0 if h < 2 else po1
                    nc.vector.scalar_tensor_tensor(
                        ot[:, ds(h * 256, 256)], po[:, ds((h % 2) * 256, 256)],
                        recip[:], bv_t[:, ds(h * 256, 256)],
                        OP.mult, OP.add)
                    nc.sync.dma_start(out_d[ts(qt, P), ds(h * 256, 256)],
                                      ot[:, ds(h * 256, 256)])

    nc.compile()
    return nc


_NC = None


def _get_nc():
    global _NC
    if _NC is None:
        _NC = build()
    return _NC


def _install_profile_hook():
    """The agent image's `antenv` lacks `axon_hooks`, so the boot-time NTFF
    profile hook install degrades silently. Recreate the registry module and
    install the ctypes-based hook so trace=True yields exec_time_ns."""
    import types
    try:
        from antenv.axon_hooks import get_axon_ntff_profile_hook  # noqa: F401
        return  # already present
    except ImportError:
        pass
    import antenv
    mod = types.ModuleType("antenv.axon_hooks")
    _hook = [None]
    mod.set_axon_ntff_profile_hook = lambda h: _hook.__setitem__(0, h)
    mod.get_axon_ntff_profile_hook = lambda: _hook[0]
    sys.modules["antenv.axon_hooks"] = mod
    antenv.axon_hooks = mod
    sys.path.insert(0, "/root/.axon_site")
    from trn_agent_boot.trn_boot import _ntff_profile_via_ctypes
    mod.set_axon_ntff_profile_hook(
        _ntff_profile_via_ctypes("/opt/axon/libaxon_pjrt.so"))


def _prep_in_maps(inputs):
    f32 = np.float32
    q = np.asarray(inputs["query"], f32)
    k = np.asarray(inputs["key"], f32)
    v = np.asarray(inputs["value"], f32)

    def do_major(w):  # [D, D] -> [do, p, (di c)]: 2KB-contiguous rows
        w4 = w.astype(NPDT).reshape(DJ, P, DJ, P)  # [di, p, do, c]
        return np.ascontiguousarray(
            w4.transpose(2, 1, 0, 3).reshape(DJ, P, D))

    wq = do_major(np.asarray(inputs["wq"], f32))
    wk = do_major(np.asarray(inputs["wk"], f32))
    wv = np.ascontiguousarray(np.asarray(inputs["wv"], f32).astype(NPDT))
    bq = np.ascontiguousarray(np.asarray(inputs["bq"], f32).reshape(DJ, P).T)
    bk = np.ascontiguousarray(np.asarray(inputs["bk"], f32).reshape(DJ, P).T)
    bv = np.ascontiguousarray(
        np.broadcast_to(np.asarray(inputs["bv"], f32).astype(NPDT), (P, D)))

    in_maps = []
    for c in range(NCORES):
        b, h = divmod(c, 2)
        sl = slice(h * QH, (h + 1) * QH)
        qT = np.ascontiguousarray(q[b, sl, :].astype(NPDT).T)
        kT = np.ascontiguousarray(k[b, sl, :].astype(NPDT).T)
        vT = np.ascontiguousarray(v[b, sl, :].astype(NPDT).T)
        in_maps.append({
            "qT": qT, "kT": kT, "vT": vT,
            "wq": wq, "wk": wk, "wv": wv,
            "bqc": bq, "bkc": bk, "bvb": bv,
        })
    return in_maps


def run(inputs, trace=False):
    """Returns (full_output [B,S,D] fp32, exec_time_ns or None)."""
    nc = _get_nc()
    in_maps = _prep_in_maps(inputs)
    if trace:
        _install_profile_hook()
    res = run_bass_kernel_spmd(nc, in_maps, list(range(NCORES)), trace=trace)
    out = np.empty((B, S, D), np.float32)
    for c in range(NCORES):
        b, h = divmod(c, 2)
        out[b, h * QH:(h + 1) * QH, :] = np.asarray(
            res.results[c]["out"]).astype(np.float32)
    return out, res.exec_time_ns


def kernel(**inputs):
    return run(inputs, trace=False)[0]


# revision 13
# speedup vs baseline: 1.0095x; 1.0095x over previous
"""Single-head attention (B=4, S=2048, D=1024) on 8 TRN2 NeuronCores.

Sharding: each core handles one (batch, query-half) pair -> 8 shards of
1024 query rows. K/V projections are split between the two cores of a
batch pair (each projects its own 1024-row sequence half) and exchanged
with 2-rank AllGathers, overlapped with the V/Q projections.

v2 pipeline changes vs baseline:
  - PE warm-up: dummy matmuls issued at t=0 fill the input-DMA head so the
    HAM clock gate reaches 8/8 before real matmuls arrive.
  - projection order K -> V -> Q: V-gathers trigger early so the gathered V
    lands before the AV phase needs it; scores (which need gathered K) run
    after Q proj anyway.
  - K is exchanged in fp8e4 (half the wire bytes) because the scores matmul
    consumes it in fp8 anyway; 1 chunk. V stays bf16, 4 chunks.
  - gpsimd is a pure collective queue (collective_compute only exists
    there); bounce writes go on scalar, reloads on sync, so collective
    completion waits never block evictions/compute.
  - scores matmul in fp8 DoubleRow (2 contraction tiles per instruction):
    q_proj/k_proj quantized at natural scale (sigma ~0.6), the 1/32
    attention scale is folded into the exp() activation's scale operand.
  - output written bf16 (host upcasts); AV k-order permuted to match
    V-gather chunk arrival.
Compute dtype bf16 (scores fp8-DR), fp32 PSUM accumulation.
"""

import sys

import numpy as np

try:
    import concourse  # noqa: F401
except ImportError:  # pragma: no cover
    sys.path.insert(0, "/opt/trn_rl_repo")

import ml_dtypes

import concourse.bass as bass  # noqa: F401
import concourse.mybir as mybir
import concourse.tile as tile
from concourse import bacc
from concourse.bass import ds, ts
from concourse.bass_utils import run_bass_kernel_spmd

P = 128          # partitions
D = 1024         # embed dim
S = 2048         # sequence length
B = 4            # batch
QH = S // 2      # query/sequence rows per core
NCORES = 8
DJ = D // P      # 8  d-tiles
KJ = S // P      # 16 k/s-tiles
HJ = KJ // 2     # 8  s-tiles per half
QJ = QH // P     # 8  q-tiles
NCH = 512        # moving-operand chunk (one PSUM bank of fp32)
ISCALE = 1.0 / 32.0  # 1/sqrt(D), folded into the exp activation
NWARM = 10       # PE warm-up matmuls (~3.5us busy: HAM hot by then)

DT = mybir.dt.bfloat16
F8 = mybir.dt.float8e4
F32 = mybir.dt.float32
NPDT = ml_dtypes.bfloat16

AF = mybir.ActivationFunctionType
OP = mybir.AluOpType
DR = mybir.MatmulPerfMode.DoubleRow

PAIRS = [[0, 1], [2, 3], [4, 5], [6, 7]]

# AV contraction order: V-gather chunk c delivers global k-tiles
# {2c, 2c+1, 8+2c, 8+2c+1}; consume in arrival order.
KT_ORDER = [kt for c in range(HJ // 2)
            for kt in (2 * c, 2 * c + 1, HJ + 2 * c, HJ + 2 * c + 1)]


def build():
    nc = bacc.Bacc("TRN2", target_bir_lowering=False, debug=False,
                   num_devices=NCORES)

    qT_d = nc.dram_tensor("qT", [D, QH], DT, kind="ExternalInput").ap()
    kT_d = nc.dram_tensor("kT", [D, QH], DT, kind="ExternalInput").ap()
    vT_d = nc.dram_tensor("vT", [D, QH], DT, kind="ExternalInput").ap()
    # wq/wk pre-sliced by output tile on the host: [do, d_in, 128]
    wq_d = nc.dram_tensor("wq", [DJ, P, D], DT, kind="ExternalInput").ap()
    wk_d = nc.dram_tensor("wk", [DJ, P, D], DT, kind="ExternalInput").ap()
    wv_d = nc.dram_tensor("wv", [D, D], DT, kind="ExternalInput").ap()
    bq_d = nc.dram_tensor("bqc", [P, DJ], F32, kind="ExternalInput").ap()
    bk_d = nc.dram_tensor("bkc", [P, DJ], F32, kind="ExternalInput").ap()
    bv_d = nc.dram_tensor("bvb", [P, D], DT, kind="ExternalInput").ap()
    out_d = nc.dram_tensor("out", [QH, D], DT, kind="ExternalOutput").ap()

    def part3(ap):  # [(n p), d] -> [p, n, d]
        return ap.rearrange("(n p) d -> p n d", p=P)

    with tile.TileContext(nc) as tc:
        with (
            tc.tile_pool(name="persist", bufs=1) as pp,
            tc.tile_pool(name="xin", bufs=16) as xp,
            tc.tile_pool(name="win", bufs=8) as wp,
            tc.tile_pool(name="ev", bufs=3) as ep,
            tc.tile_pool(name="psum", bufs=3, space="PSUM") as psp,
            tc.tile_pool(name="dram", bufs=1, space="DRAM") as dp,
        ):
            # collective bounce buffers (internal DRAM)
            KCH = 8  # K gather: one fp8 collective (1MB send)
            kbc = [dp.tile([KCH * P, QH], F8, tag=f"kb{c}", name=f"kb{c}")
                   for c in range(DJ // KCH)]
            kgc = [dp.tile([2, KCH * P, QH], F8, tag=f"kg{c}", name=f"kg{c}")
                   for c in range(DJ // KCH)]
            vbc = [dp.tile([2 * P, D], DT, tag=f"vb{c}", name=f"vb{c}")
                   for c in range(HJ // 2)]
            vgc = [dp.tile([2, 2 * P, D], DT, tag=f"vg{c}", name=f"vg{c}")
                   for c in range(HJ // 2)]

            # constants + warm-up scratch (vector queue)
            ones_t = pp.tile([P, 1], DT, tag="ones")
            nc.vector.memset(ones_t[:], 1.0)
            warm_t = pp.tile([P, NCH], DT, tag="warm")
            nc.vector.memset(warm_t[:], 0.25)

            # PE warm-up: garbage matmuls with no input deps beyond the memset
            for i in range(NWARM):
                psw = psp.tile([P, NCH], F32, tag="psA")
                nc.tensor.matmul(psw[:], warm_t[:, ds(0, P)], warm_t[:],
                                 start=True, stop=True)

            # ---- all input/weight loads issued up front
            # sync queue: x inputs (K first, then V, then Q), then K gathers
            def load_x(src, tag="x", split=0, eng=None, stripe=False):
                eng = eng or nc.sync
                out = []
                for di in range(DJ):
                    t = xp.tile([P, QH], DT, tag=tag)
                    # striped halves: second ring doubles the engine pull;
                    # gpsimd's ring is idle until the first collective (~55us)
                    e2 = nc.gpsimd if stripe else eng
                    if di < split:
                        eng.dma_start(t[:, ds(0, NCH)],
                                      src[:, di, ds(0, NCH)])
                        e2.dma_start(t[:, ds(NCH, NCH)],
                                     src[:, di, ds(NCH, NCH)])
                    else:
                        eng.dma_start(t[:], src[:, di, :])
                    out.append(t)
                return out

            kT_in = load_x(part3(kT_d), tag="xk", split=DJ, stripe=True)
            vT_in = load_x(part3(vT_d))
            # qT loads are emitted after the K-projection loop so the K
            # bounce-eviction DMAs aren't queued behind them on the sync ring

            # scalar queue: weights (K first, then V, then Q)
            def load_w2(src):
                out = []
                for do in range(DJ):
                    t = wp.tile([P, DJ, P], DT, tag="w")
                    if do == 0:
                        nc.scalar.dma_start(t[:, ds(0, DJ // 2), :],
                                            src[do][:, ds(0, D // 2)])
                        nc.scalar.dma_start(t[:, ds(DJ // 2, DJ // 2), :],
                                            src[do][:, ds(D // 2, D // 2)])
                    else:
                        nc.scalar.dma_start(t[:], src[do])
                    out.append(t)
                return out

            bk_t = pp.tile([P, DJ], F32, tag="bk")
            nc.scalar.dma_start(bk_t[:], bk_d[:])
            bq_t = pp.tile([P, DJ], F32, tag="bq")
            nc.scalar.dma_start(bq_t[:], bq_d[:])
            wk_t = load_w2(wk_d)

            def load_w(src):
                out = []
                for di in range(DJ):
                    t = wp.tile([P, D], DT, tag="wv")
                    nc.scalar.dma_start(t[:], src[:, di, :])
                    out.append(t)
                return out

            wv_t = load_w(part3(wv_d))


            # persistent intermediates
            qT_proj = pp.tile([P, DJ, QH], F8, tag="qproj")   # (Q+bq)^T
            expT = pp.tile([P, KJ, QH], DT, tag="expT")       # exp(scores)^T
            v_full = pp.tile([P, KJ, D], DT, tag="vfull")     # gathered V
            # gathered K^T in fp8, grouped by do-PAIR for DoubleRow:
            # kf[p] = [128, 2, S] covering do = 2p, 2p+1
            kT_f = [pp.tile([P, 2, S], F8, tag=f"kf{p}", name=f"kf{p}")
                    for p in range(DJ // 2)]

            # ---- K projection (own half first; evict fp8 -> bounce)
            for do in range(DJ):
                ps0 = psp.tile([P, NCH], F32, tag="psA")
                ps1 = psp.tile([P, NCH], F32, tag="psB")
                for di in range(DJ):
                    w_ap = wk_t[do][:, di, :]
                    nc.tensor.matmul(ps0[:], w_ap, kT_in[di][:, ds(0, NCH)],
                                     start=(di == 0), stop=(di == DJ - 1))
                    nc.tensor.matmul(ps1[:], w_ap, kT_in[di][:, ds(NCH, NCH)],
                                     start=(di == 0), stop=(di == DJ - 1))
                ev = ep.tile([P, D], F8, tag="evk", bufs=4)
                nc.vector.tensor_scalar_add(ev[:, ds(0, NCH)], ps0[:],
                                            bk_t[:, ds(do, 1)])
                nc.vector.tensor_scalar_add(ev[:, ds(NCH, NCH)], ps1[:],
                                            bk_t[:, ds(do, 1)])
                nc.sync.dma_start(kbc[do // KCH][ts(do % KCH, P), :], ev[:])
                if do % KCH == KCH - 1:
                    c = do // KCH
                    nc.gpsimd.collective_compute(
                        "AllGather", OP.bypass, replica_groups=PAIRS,
                        ins=[kbc[c].opt()], outs=[kgc[c].opt()])
                    # reload right behind the gather on the same queue
                    for dd in range(KCH):
                        dog = c * KCH + dd
                        for g in range(2):
                            nc.sync.dma_start(
                                kT_f[dog // 2][:, dog % 2, ds(g * QH, QH)],
                                kgc[c][g, ts(dd, P), :])

            qT_in = load_x(part3(qT_d), eng=nc.scalar)

            # ---- V projection (own half, natural layout, no bias)
            for st in range(HJ):
                ps0 = psp.tile([P, NCH], F32, tag="psA")
                ps1 = psp.tile([P, NCH], F32, tag="psB")
                for di in range(DJ):
                    v_ap = vT_in[di][:, ts(st, P)]
                    nc.tensor.matmul(ps0[:], v_ap, wv_t[di][:, ds(0, NCH)],
                                     start=(di == 0), stop=(di == DJ - 1))
                    nc.tensor.matmul(ps1[:], v_ap, wv_t[di][:, ds(NCH, NCH)],
                                     start=(di == 0), stop=(di == DJ - 1))
                ev = ep.tile([P, D], DT, tag="ev")
                nc.vector.tensor_copy(ev[:, ds(0, NCH)], ps0[:])
                nc.vector.tensor_copy(ev[:, ds(NCH, NCH)], ps1[:])
                nc.scalar.dma_start(vbc[st // 2][ts(st % 2, P), :], ev[:])
                if st % 2 == 1:
                    c = st // 2
                    nc.gpsimd.collective_compute(
                        "AllGather", OP.bypass, replica_groups=PAIRS,
                        ins=[vbc[c].opt()], outs=[vgc[c].opt()])
                    # reload the 4 global k-tiles this chunk delivers
                    for g in range(2):
                        for j in range(2):
                            kt = g * HJ + 2 * c + j
                            nc.sync.dma_start(v_full[:, kt, :],
                                              vgc[c][g, ts(j, P), :])

            # wq loads reuse wk's pool slots (freed ~36us, needed ~63us);
            # emitted here so the bounce-eviction DMAs above aren't stuck
            # behind their WAR waits on the scalar queue
            wq_t = load_w2(wq_d)
            bv_t = pp.tile([P, D], DT, tag="bv")
            nc.scalar.dma_start(bv_t[:], bv_d[:])

            # ---- Q projection -> qT_proj fp8 [d_out, q] (natural scale)
            for do in range(DJ):
                ps0 = psp.tile([P, NCH], F32, tag="psA")
                ps1 = psp.tile([P, NCH], F32, tag="psB")
                for di in range(DJ):
                    w_ap = wq_t[do][:, di, :]
                    nc.tensor.matmul(ps0[:], w_ap, qT_in[di][:, ds(0, NCH)],
                                     start=(di == 0), stop=(di == DJ - 1))
                    nc.tensor.matmul(ps1[:], w_ap, qT_in[di][:, ds(NCH, NCH)],
                                     start=(di == 0), stop=(di == DJ - 1))
                nc.vector.tensor_scalar_add(qT_proj[:, do, ds(0, NCH)],
                                            ps0[:], bq_t[:, ds(do, 1)])
                nc.vector.tensor_scalar_add(qT_proj[:, do, ds(NCH, NCH)],
                                            ps1[:], bq_t[:, ds(do, 1)])

            # ---- scores^T + exp -> expT [k, q]  (fp8 DoubleRow)
            for kt in range(KJ):
                ps0 = psp.tile([P, NCH], F32, tag="psA")
                ps1 = psp.tile([P, NCH], F32, tag="psB")
                for c in range(DJ // 2):
                    k_ap = kT_f[c][:, :, ts(kt, P)]
                    nc.tensor.matmul(ps0[:], k_ap,
                                     qT_proj[:, ds(2 * c, 2), ds(0, NCH)],
                                     start=(c == 0), stop=(c == DJ // 2 - 1),
                                     perf_mode=DR)
                    nc.tensor.matmul(ps1[:], k_ap,
                                     qT_proj[:, ds(2 * c, 2), ds(NCH, NCH)],
                                     start=(c == 0), stop=(c == DJ // 2 - 1),
                                     perf_mode=DR)
                nc.scalar.activation(expT[:, kt, ds(0, NCH)], ps0[:], AF.Exp,
                                     scale=ISCALE)
                nc.scalar.activation(expT[:, kt, ds(NCH, NCH)], ps1[:],
                                     AF.Exp, scale=ISCALE)

            # ---- AV + fused normalize/bias -> out (bf16)
            for qt in range(QJ):
                po0 = psp.tile([P, NCH], F32, tag="psA")
                po1 = psp.tile([P, NCH], F32, tag="psB")
                psm = psp.tile([P, 1], F32, tag="psS", bufs=2)
                for i, kt in enumerate(KT_ORDER):
                    e_ap = expT[:, kt, ts(qt, P)]
                    nc.tensor.matmul(po0[:], e_ap, v_full[:, kt, ds(0, NCH)],
                                     start=(i == 0), stop=(i == KJ - 1))
                    nc.tensor.matmul(po1[:], e_ap, v_full[:, kt, ds(NCH, NCH)],
                                     start=(i == 0), stop=(i == KJ - 1))
                    nc.tensor.matmul(psm[:], e_ap, ones_t[:],
                                     start=(i == 0), stop=(i == KJ - 1))
                recip = ep.tile([P, 1], F32, tag="recip")
                nc.vector.reciprocal(recip[:], psm[:])
                ot = ep.tile([P, D], DT, tag="out", bufs=2)
                for h in range(4):
                    po = po0 if h < 2 else po1
                    nc.vector.scalar_tensor_tensor(
                        ot[:, ds(h * 256, 256)], po[:, ds((h % 2) * 256, 256)],
                        recip[:], bv_t[:, ds(h * 256, 256)],
                        OP.mult, OP.add)
                    nc.sync.dma_start(out_d[ts(qt, P), ds(h * 256, 256)],
                                      ot[:, ds(h * 256, 256)])

    nc.compile()
    return nc


_NC = None


def _get_nc():
    global _NC
    if _NC is None:
        _NC = build()
    return _NC


def _install_profile_hook():
    """The agent image's `antenv` lacks `axon_hooks`, so the boot-time NTFF
    profile hook install degrades silently. Recreate the registry module and
    install the ctypes-based hook so trace=True yields exec_time_ns."""
    import types
    try:
        from antenv.axon_hooks import get_axon_ntff_profile_hook  # noqa: F401
        return  # already present
    except ImportError:
        pass
    import antenv
    mod = types.ModuleType("antenv.axon_hooks")
    _hook = [None]
    mod.set_axon_ntff_profile_hook = lambda h: _hook.__setitem__(0, h)
    mod.get_axon_ntff_profile_hook = lambda: _hook[0]
    sys.modules["antenv.axon_hooks"] = mod
    antenv.axon_hooks = mod
    sys.path.insert(0, "/root/.axon_site")
    from trn_agent_boot.trn_boot import _ntff_profile_via_ctypes
    mod.set_axon_ntff_profile_hook(
        _ntff_profile_via_ctypes("/opt/axon/libaxon_pjrt.so"))


def _prep_in_maps(inputs):
    f32 = np.float32
    q = np.asarray(inputs["query"], f32)
    k = np.asarray(inputs["key"], f32)
    v = np.asarray(inputs["value"], f32)

    def do_major(w):  # [D, D] -> [do, p, (di c)]: 2KB-contiguous rows
        w4 = w.astype(NPDT).reshape(DJ, P, DJ, P)  # [di, p, do, c]
        return np.ascontiguousarray(
            w4.transpose(2, 1, 0, 3).reshape(DJ, P, D))

    wq = do_major(np.asarray(inputs["wq"], f32))
    wk = do_major(np.asarray(inputs["wk"], f32))
    wv = np.ascontiguousarray(np.asarray(inputs["wv"], f32).astype(NPDT))
    bq = np.ascontiguousarray(np.asarray(inputs["bq"], f32).reshape(DJ, P).T)
    bk = np.ascontiguousarray(np.asarray(inputs["bk"], f32).reshape(DJ, P).T)
    bv = np.ascontiguousarray(
        np.broadcast_to(np.asarray(inputs["bv"], f32).astype(NPDT), (P, D)))

    in_maps = []
    for c in range(NCORES):
        b, h = divmod(c, 2)
        sl = slice(h * QH, (h + 1) * QH)
        qT = np.ascontiguousarray(q[b, sl, :].astype(NPDT).T)
        kT = np.ascontiguousarray(k[b, sl, :].astype(NPDT).T)
        vT = np.ascontiguousarray(v[b, sl, :].astype(NPDT).T)
        in_maps.append({
            "qT": qT, "kT": kT, "vT": vT,
            "wq": wq, "wk": wk, "wv": wv,
            "bqc": bq, "bkc": bk, "bvb": bv,
        })
    return in_maps


def run(inputs, trace=False):
    """Returns (full_output [B,S,D] fp32, exec_time_ns or None)."""
    nc = _get_nc()
    in_maps = _prep_in_maps(inputs)
    if trace:
        _install_profile_hook()
    res = run_bass_kernel_spmd(nc, in_maps, list(range(NCORES)), trace=trace)
    out = np.empty((B, S, D), np.float32)
    for c in range(NCORES):
        b, h = divmod(c, 2)
        out[b, h * QH:(h + 1) * QH, :] = np.asarray(
            res.results[c]["out"]).astype(np.float32)
    return out, res.exec_time_ns


def kernel(**inputs):
    return run(inputs, trace=False)[0]
